# revision 36
# baseline (speedup 1.0000x reference)
"""Masked multi-head attention (B=8, N=1024, C=768, H=12) on 8 TRN2 NeuronCores.

Data-parallel: one batch element per core, no collectives.

Load-bearing ideas on top of the usual transposed-attention layout:

1. Key compaction. The mask kills key positions for ALL queries and heads,
   so the host gathers the unmasked key positions (padded to a multiple of
   128 with dead keys whose exp underflows to 0 via the -60000 bias) and
   the kernel only runs scores/exp/attn@v over NTK = ceil(max_keys/128)
   key tiles instead of 8. Queries are never masked, so q stays full-width.
   The kernel is compiled per NTK (cached); for the target inputs NTK=5.

2. All input massaging happens on the host: x arrives pre-transposed,
   pre-cast to bf16, partition-major contiguous (so every DMA is 128 big
   descriptors), and split in token halves; w_qkv arrives bf16
   column-blocked per (pair, k/q/v) unit and chunked per pair so pair 0's
   weights land first. Startup is HBM-bandwidth-bound, so the first
   attention pass needs only the first halves.

3. Two-pass attention per pair (q-half 0, then q-half 1): the qc0
   normalizer pipeline hides completely under the qc1 pass, and pair 0's
   pass A can start before the h1 DMAs land.

4. The softmax normalizer broadcast runs as a DRAM bounce for pairs with
   following work to hide it, but as a 1-row PE matmul against a ones
   column for the LAST pair, where the DMA latency would gate the output
   projection.

5. The output is written bf16 (halves the drain DMA) and cast back to
   f32 on the host.

The next pair's qkv units are software-pipelined into the current pair's
key-tile loops as PE filler; for the last pair the filler is the first
half of the output projection (which only needs every pair's qc0 attnT).

Matmuls run in bf16 (f32 PSUM accumulation). Built on Bacc so matmul
sync waits get legalized.
"""

import numpy as np
from collections import deque
from contextlib import ExitStack

import ml_dtypes

import concourse.bass as bass
import concourse.tile as tile
from concourse import bacc, mybir
from concourse.bass_utils import run_bass_kernel_spmd
from concourse.masks import make_identity

F32 = mybir.dt.float32
BF16 = mybir.dt.bfloat16
FP8 = mybir.dt.float8e4
I32 = mybir.dt.int32
AF = mybir.ActivationFunctionType
ALU = mybir.AluOpType

B = 8
N = 1024          # tokens
C = 768           # channels
H = 12            # heads
DH = 64           # head dim
P = 128           # partitions
KT = C // P       # 6 contraction tiles over C
NPAIR = H // 2    # 6 head pairs (2 heads per 128-partition tile)
SCALE = DH ** -0.5
MASK_NEG = -60000.0
NCORES = 8
BF = ml_dtypes.bfloat16


def _body(ctx, tc, ntk, xT_ext, xgT_ext, maskb_ext, wqkv_ext, wproj_ext,
          bproj_ext, out_ext):
    nc = tc.nc
    KP = ntk * P
    KP2 = KP // 2

    singles = ctx.enter_context(tc.tile_pool(name="singles", bufs=1))
    qkv_pool = ctx.enter_context(tc.tile_pool(name="qkv", bufs=3))
    pt_pool = ctx.enter_context(tc.tile_pool(name="pt", bufs=18))
    zb_pool = ctx.enter_context(tc.tile_pool(name="zb", bufs=2))
    out_pool = ctx.enter_context(tc.tile_pool(name="outp", bufs=2))
    ps_pool = ctx.enter_context(tc.tile_pool(name="ps", bufs=3, space="PSUM"))
    ps_av = ctx.enter_context(tc.tile_pool(name="ps_av", bufs=2, space="PSUM"))
    dram_pool = ctx.enter_context(tc.tile_pool(name="dram", bufs=2, space="DRAM"))

    # ---- startup-critical DMAs first: pair-0 weights and the h0 x-halves
    # race on separate queues; bias_bc (only needed at projection time)
    # and the later weight chunks follow ----
    maskb = singles.tile([P, ntk], F32)
    nc.sync.dma_start(out=maskb[:], in_=maskb_ext.rearrange("i p -> p i"))

    # each x half lands as two k-sub-chunks so the qkv units' k-loops can
    # start streaming after ~0.25MB instead of a full half
    xgT = singles.tile([P, 2, KT, KP2], BF16)
    xgT_v = xgT_ext.rearrange("p (h k n) -> p h k n", h=2, k=KT)
    for h in range(2):
        for c in range(2):
            nc.scalar.dma_start(out=xgT[:, h, 3 * c:3 * c + 3],
                                in_=xgT_v[:, h, 3 * c:3 * c + 3])
    xT = singles.tile([P, 2, KT, 512], BF16)
    xT_v = xT_ext.rearrange("p (h k n) -> p h k n", h=2, k=KT)
    for h in range(2):
        for c in range(2):
            nc.gpsimd.dma_start(out=xT[:, h, 3 * c:3 * c + 3],
                                in_=xT_v[:, h, 3 * c:3 * c + 3])

    # weight chunks spread across all three queues by need-time so none
    # starves behind the x transfers: sync gets the early pairs, the x
    # queues pick up the later pairs once their halves are done
    wqkv_b = singles.tile([P, 3 * NPAIR, KT, P], BF16)
    wqkv_v = wqkv_ext.rearrange("p (m k j) -> p m k j", m=3 * NPAIR, k=KT)
    w_queue = {0: nc.sync, 1: nc.sync, 2: nc.sync,
               3: nc.scalar, 4: nc.gpsimd, 5: nc.scalar}
    for g in range(NPAIR):
        if g == 0:
            # pair 0 per-unit (k first): the very first matmul only waits
            # on the 0.2MB k-weight slice
            for m in range(3):
                nc.sync.dma_start(out=wqkv_b[:, m:m + 1],
                                  in_=wqkv_v[:, m:m + 1])
        else:
            w_queue.get(g, nc.sync).dma_start(
                out=wqkv_b[:, 3 * g:3 * g + 3], in_=wqkv_v[:, 3 * g:3 * g + 3])
    bias_bc = singles.tile([P, C], F32)
    nc.gpsimd.dma_start(out=bias_bc[:],
                        in_=bproj_ext[0:1, :].to_broadcast([P, C]))
    wproj_sb = singles.tile([P, KT, C], BF16)
    nc.scalar.dma_start(
        out=wproj_sb[:], in_=wproj_ext.rearrange("p (k n) -> p k n", k=KT))

    ident_f = singles.tile([P, P], F32)
    make_identity(nc, ident_f[:])
    ident = singles.tile([P, P], BF16)
    nc.vector.tensor_copy(out=ident[:], in_=ident_f[:])
    zsel = singles.tile([P, DH], F32)
    nc.vector.memset(zsel[:], 0.0)
    nc.vector.memset(zsel[DH:DH + 1, :], 1.0)

    attnT = singles.tile([P, KT, N], BF16)

    # persistent [v | 1] tiles: ones column written once
    VE = DH + 1
    ve_tiles = [singles.tile([P, 2, VE], BF16, name=f"ve_s{i}")
                for i in range(ntk)]
    for t_ in ve_tiles:
        nc.vector.memset(t_[:, :, DH:DH + 1], 1.0)

    # ---- qkv projection units: u//2 -> (k, q, v), u%2 -> token half ----
    qkv_tiles = {}

    def qkv_unit(p, u):
        t, name = ((0, "k"), (1, "q"), (2, "v"))[u // 2]
        half = u % 2
        cols = N if name == "q" else KP
        w = cols // 2
        if (p, name) not in qkv_tiles:
            qkv_tiles[(p, name)] = qkv_pool.tile(
                [P, cols], BF16, tag=name, name=f"{name}{p}")
        dst = qkv_tiles[(p, name)]
        src = xT if name == "q" else xgT
        ps = ps_pool.tile([P, w], F32, tag="ps", name=f"ps_{name}{p}_{half}")
        for k in range(KT):
            nc.tensor.matmul(
                out=ps[:],
                lhsT=wqkv_b[:, 3 * p + t, k, :],
                rhs=src[:, half, k, :],
                start=(k == 0), stop=(k == KT - 1))
        nc.vector.tensor_copy(out=dst[:, half * w:(half + 1) * w], in_=ps[:])

    # pair 0's h0 units (and the h1 units pass A will need) run upfront;
    # everything else trickles in as PE filler via the unit fifo
    unit_fifo = deque()
    for u in (0, 4, 2, 1, 5):
        qkv_unit(0, u)
    unit_fifo.append((0, 3))

    def normalize(p, qc, av_sbs):
        # attnT[:, p, qc-half] = av * (1/normalizer-row)
        for hi in range(2):
            av_sb = av_sbs[hi]
            zrow = av_sb[DH:DH + 1, qc * 512:(qc + 1) * 512]
            if p == NPAIR - 1:
                # PE broadcast: fan the raw normalizer row out with a
                # selector matmul (row 64 of zsel is 1), then recip the
                # broadcast at partition 0 — no DMA on the critical path.
                # (reciprocal_approx_fast miscomputes at partition offset
                # 64, so the recip must come after the broadcast.)
                zps = ps_pool.tile([DH, 512], F32, tag="ps",
                                   name=f"zps{p}_{hi}_{qc}")
                nc.tensor.matmul(
                    out=zps[:], lhsT=zsel[0:DH + 1, :],
                    rhs=av_sb[0:DH + 1, qc * 512:(qc + 1) * 512],
                    start=True, stop=True)
                zb = zb_pool.tile([DH, 512], F32, tag="zb",
                                  name=f"zbp{p}_{hi}_{qc}")
                nc.vector.reciprocal_approx_fast(out=zb[:], in_=zps[:])
                zin = zb[:]
            else:
                # DRAM bounce, hidden under the following pass's PE work
                zdram = dram_pool.tile([1, 512], F32, tag="zdram",
                                       name=f"zd{p}_{hi}_{qc}")
                nc.sync.dma_start(out=zdram[:], in_=zrow)
                zb = zb_pool.tile([DH, 512], F32, tag="zb",
                                  name=f"zb{p}_{hi}_{qc}")
                nc.sync.dma_start(out=zb[:],
                                  in_=zdram[0:1, :].to_broadcast([DH, 512]))
                nc.vector.reciprocal_approx_fast(out=zb[:], in_=zb[:])
                zin = zb[:]
            nc.vector.scalar_tensor_tensor(
                out=attnT[64 * hi:64 * (hi + 1), p, qc * 512:(qc + 1) * 512],
                in0=av_sb[0:DH, qc * 512:(qc + 1) * 512], scalar=1.0, in1=zin,
                op0=ALU.mult, op1=ALU.mult)

    out_queues = [nc.sync, nc.scalar, nc.gpsimd]

    def proj_wave(m0, m1):
        # out rows m*128..: needs attnT q-columns m*128.. only
        chunks = [(0, 512), (512, 256)]
        for m in range(m0, m1):
            out_sb = out_pool.tile([P, C], BF16, tag="out_sb", name=f"out_sb{m}")
            for j, (lo, w) in enumerate(chunks):
                pps = ps_pool.tile([P, w], F32, tag="ps", name=f"ps_proj{m}_{j}")
                for k in range(KT):
                    nc.tensor.matmul(
                        out=pps[:],
                        lhsT=attnT[:, k, m * P:(m + 1) * P],
                        rhs=wproj_sb[:, k, lo:lo + w],
                        start=(k == 0), stop=(k == KT - 1))
                nc.vector.scalar_tensor_tensor(
                    out=out_sb[:, lo:lo + w], in0=pps[:], scalar=1.0,
                    in1=bias_bc[:, lo:lo + w], op0=ALU.mult, op1=ALU.add)
            out_queues[m % 3].dma_start(
                out=out_ext[m * P:(m + 1) * P, :], in_=out_sb[:])

    # ---- per head pair: two passes (q-half 0, then q-half 1) ----
    for p in range(NPAIR):
        if p + 1 < NPAIR:
            for u in (0, 2, 4, 1, 5, 3):
                unit_fifo.append((p + 1, u))
        nslots = 2 * ntk
        pending = len(unit_fifo)

        qt = qkv_tiles[(p, "q")]
        kt_ = qkv_tiles[(p, "k")]
        vt = qkv_tiles[(p, "v")]

        av = [ps_av.tile([VE, 512], F32, tag="ps_av", name=f"av{p}_{hi}")
              for hi in range(2)]
        av_sbs = [zb_pool.tile([VE, N], F32, tag="av_sb", name=f"avs{p}_{hi}")
                  for hi in range(2)]

        # this pair's own deferred units must precede the loop that
        # reads their output
        while unit_fifo and unit_fifo[0][0] == p:
            qkv_unit(*unit_fifo.popleft())

        all_pts = [[None, None] for _ in range(ntk)]
        # pair 0 runs q-half 0 for every key tile before touching q-half 1:
        # its pass A needs only the h0 DMAs, so attention starts ~5 us
        # earlier; later pairs interleave halves (better steady cadence)
        if p == 0:
            order = [(kb, 0) for kb in range(ntk)] + \
                    [(kb, 1) for kb in range(ntk)]
        else:
            order = [(kb, qc) for kb in range(ntk) for qc in range(2)]
        for kb, qc in order:
            if True:
                psq = ps_pool.tile([P, N], F32, tag="ps", name=f"ps_s{p}_{kb}_{qc}")
                for hi in range(2):
                    nc.tensor.matmul(
                        out=psq[:, 512 * hi:512 * (hi + 1)],
                        lhsT=kt_[64 * hi:64 * (hi + 1), kb * P:(kb + 1) * P],
                        rhs=qt[64 * hi:64 * (hi + 1), qc * 512:(qc + 1) * 512],
                        start=True, stop=True)
                pt = pt_pool.tile([P, N], BF16, tag="pt", name=f"pt{p}_{kb}_{qc}")
                nc.scalar.activation(
                    out=pt[:], in_=psq[:], func=AF.Exp,
                    bias=maskb[:, kb:kb + 1], scale=SCALE)
                all_pts[kb][qc] = pt

            if qc == 0:
                # v natural block for this key tile (needed later, so it
                # sits behind the scores on the PE queue)
                vnat = ps_pool.tile([P, P], BF16, tag="ps", name=f"vn{p}_{kb}")
                nc.tensor.transpose(
                    out=vnat[:], in_=vt[:, kb * P:(kb + 1) * P],
                    identity=ident[:])
                nc.vector.tensor_copy(
                    out=ve_tiles[kb][:, :, 0:DH],
                    in_=vnat[:].rearrange("p (h d) -> p h d", h=2))

                # previous key tile's qc0 p@v: slot-free PE work while
                # exps run
                if kb > 0:
                    for hi in range(2):
                        nc.tensor.matmul(
                            out=av[hi][:],
                            lhsT=ve_tiles[kb - 1][:, hi, :],
                            rhs=all_pts[kb - 1][0][:, 512 * hi:512 * (hi + 1)],
                            start=(kb - 1 == 0), stop=False)

            # PE filler while exps run: queued qkv units, spread evenly
            # (pair 0's loop stays clean — the queue is in-order and
            # fillers there would stall it on not-yet-landed h1 DMAs)
            if p > 0 and qc == 1:
                want = (pending * (kb + 1) + ntk - 1) // ntk
                while pending - len(unit_fifo) < want and unit_fifo:
                    qkv_unit(*unit_fifo.popleft())

        # close qc0 with the last key tile, copy out, normalize half 0
        for hi in range(2):
            nc.tensor.matmul(
                out=av[hi][:], lhsT=ve_tiles[ntk - 1][:, hi, :],
                rhs=all_pts[ntk - 1][0][:, 512 * hi:512 * (hi + 1)],
                start=(ntk == 1), stop=True)
        for hi in range(2):
            nc.vector.tensor_copy(out=av_sbs[hi][:, 0:512], in_=av[hi][:])
        normalize(p, 0, av_sbs)

        # qc1 p@v as one tight pass (reuses the av psum tiles)
        for kb in range(ntk):
            for hi in range(2):
                nc.tensor.matmul(
                    out=av[hi][:],
                    lhsT=ve_tiles[kb][:, hi, :],
                    rhs=all_pts[kb][1][:, 512 * hi:512 * (hi + 1)],
                    start=(kb == 0), stop=(kb == ntk - 1))
        for hi in range(2):
            nc.vector.tensor_copy(out=av_sbs[hi][:, 512:1024], in_=av[hi][:])
        normalize(p, 1, av_sbs)

        while unit_fifo:
            qkv_unit(*unit_fifo.popleft())

        if p == NPAIR - 1:
            # projection over q 0:511 only needs every pair's qc0 attnT;
            # it fills the PE while this pair's qc1 normalizer settles
            proj_wave(0, 4)
            proj_wave(4, 8)


def build(ntk):
    nc = bacc.Bacc()
    KP = ntk * P
    xT_ext = nc.declare_dram_parameter("xT", [P, KT * N], BF16, isOutput=False)
    xgT_ext = nc.declare_dram_parameter("xgT", [P, KT * KP], BF16, isOutput=False)
    maskb_ext = nc.declare_dram_parameter("maskb", [ntk, P], F32, isOutput=False)
    wqkv_ext = nc.declare_dram_parameter(
        "w_qkv", [P, 3 * NPAIR * KT * P], BF16, isOutput=False)
    wproj_ext = nc.declare_dram_parameter("w_proj", [P, KT * C], BF16,
                                          isOutput=False)
    bproj_ext = nc.declare_dram_parameter("b_proj", [1, C], F32, isOutput=False)
    out_ext = nc.declare_dram_parameter("out", [N, C], BF16, isOutput=True)

    with tile.TileContext(nc) as tc, ExitStack() as ctx:
        _body(ctx, tc, ntk, xT_ext.ap(), xgT_ext.ap(), maskb_ext.ap(),
              wqkv_ext.ap(), wproj_ext.ap(), bproj_ext.ap(), out_ext.ap())
    nc.finalize()
    return nc


_NC_CACHE = {}


def _get_nc(ntk):
    if ntk not in _NC_CACHE:
        _NC_CACHE[ntk] = build(ntk)
    return _NC_CACHE[ntk]


def _make_in_maps(inputs):
    x = np.ascontiguousarray(np.asarray(inputs["x"], dtype=np.float32))
    mask = np.ascontiguousarray(np.asarray(inputs["mask"], dtype=np.int32))
    w_qkv = np.ascontiguousarray(np.asarray(inputs["w_qkv"], dtype=np.float32))
    w_proj = np.ascontiguousarray(np.asarray(inputs["w_proj"], dtype=np.float32))
    b_proj = np.ascontiguousarray(
        np.asarray(inputs["b_proj"], dtype=np.float32)).reshape(1, C)

    # key compaction: gather unmasked key positions, pad to a tile multiple
    idxs = [np.nonzero(mask[b] == 0)[0] for b in range(B)]
    ntk = max(1, -(-max(len(i) for i in idxs) // P))
    KP = ntk * P

    # w_qkv [C, 3C] -> [P, pair*type*kchunk*128] bf16, type order (k, q, v)
    w3 = w_qkv.reshape(KT, P, 3, NPAIR, P)[:, :, [1, 0, 2], :, :]
    wq_u = np.ascontiguousarray(
        w3.transpose(1, 3, 2, 0, 4)).reshape(P, -1).astype(BF)
    wp_u = np.ascontiguousarray(
        w_proj.reshape(KT, P, C).transpose(1, 0, 2)).reshape(P, KT * C).astype(BF)

    maps = []
    for b in range(B):
        idx = idxs[b]
        nb = len(idx)
        pad = np.zeros(KP, np.int64)
        pad[:nb] = idx  # pad slots point anywhere; their bias kills them
        maskb_h = np.full(KP, MASK_NEG, np.float32)
        maskb_h[:nb] = 0.0
        xb = x[b]
        # [P, half, kchunk, tokens-in-half], partition-major contiguous
        xT_h = xb.T.reshape(KT, P, 2, 512).transpose(1, 2, 0, 3)
        xgT_h = xb[pad].T.reshape(KT, P, 2, KP // 2).transpose(1, 2, 0, 3)
        maps.append({
            "xT": np.ascontiguousarray(xT_h).reshape(P, KT * N).astype(BF),
            "xgT": np.ascontiguousarray(xgT_h).reshape(P, KT * KP).astype(BF),
            "maskb": maskb_h.reshape(ntk, P),
            "w_qkv": wq_u,
            "w_proj": wp_u,
            "b_proj": b_proj,
        })
    return maps, ntk


def _run(inputs, trace=False, **kwargs):
    in_maps, ntk = _make_in_maps(inputs)
    nc = _get_nc(ntk)
    res = run_bass_kernel_spmd(nc, in_maps, list(range(NCORES)), trace=trace,
                               **kwargs)
    out = np.stack([np.asarray(res.results[i]["out"]).astype(np.float32)
                    for i in range(NCORES)])
    return out, res


def kernel(**inputs):
    out, _ = _run(inputs)
    return out


# revision 37
# speedup vs baseline: 1.1667x; 1.1667x over previous
"""Masked multi-head attention (B=8, N=1024, C=768, H=12) on 8 TRN2 NeuronCores.

Data-parallel: one batch element per core, no collectives.

Load-bearing ideas on top of the usual transposed-attention layout:

1. Key compaction. The mask kills key positions for ALL queries and heads,
   so the host gathers the unmasked key positions (padded to a multiple of
   128 with dead keys whose exp underflows to 0 via the -60000 bias) and
   the kernel only runs scores/exp/attn@v over NTK = ceil(max_keys/128)
   key tiles instead of 8. Queries are never masked, so q stays full-width.
   The kernel is compiled per NTK (cached); for the target inputs NTK=5.

2. All input massaging happens on the host: x arrives pre-transposed,
   pre-cast to bf16, partition-major contiguous (so every DMA is 128 big
   descriptors), and split in token halves; w_qkv arrives bf16
   column-blocked per (pair, k/q/v) unit and chunked per pair so pair 0's
   weights land first. Startup is HBM-bandwidth-bound, so the first
   attention pass needs only the first halves.

3. Two-pass attention per pair (q-half 0, then q-half 1): the qc0
   normalizer pipeline hides completely under the qc1 pass, and pair 0's
   pass A can start before the h1 DMAs land.

4. The softmax normalizer broadcast runs as a DRAM bounce for pairs with
   following work to hide it, but as a 1-row PE matmul against a ones
   column for the LAST pair, where the DMA latency would gate the output
   projection.

5. The output is written bf16 (halves the drain DMA) and cast back to
   f32 on the host.

The next pair's qkv units are software-pipelined into the current pair's
key-tile loops as PE filler; for the last pair the filler is the first
half of the output projection (which only needs every pair's qc0 attnT).

Matmuls run in bf16 (f32 PSUM accumulation). Built on Bacc so matmul
sync waits get legalized.
"""

import numpy as np
from collections import deque
from contextlib import ExitStack

import ml_dtypes

import concourse.bass as bass
import concourse.tile as tile
from concourse import bacc, mybir
from concourse.bass_utils import run_bass_kernel_spmd
from concourse.masks import make_identity

F32 = mybir.dt.float32
BF16 = mybir.dt.bfloat16
FP8 = mybir.dt.float8e4
I32 = mybir.dt.int32
AF = mybir.ActivationFunctionType
ALU = mybir.AluOpType

B = 8
N = 1024          # tokens
C = 768           # channels
H = 12            # heads
DH = 64           # head dim
P = 128           # partitions
KT = C // P       # 6 contraction tiles over C
NPAIR = H // 2    # 6 head pairs (2 heads per 128-partition tile)
SCALE = DH ** -0.5
MASK_NEG = -60000.0
NCORES = 8
BF = ml_dtypes.bfloat16


def _body(ctx, tc, ntk, xT_ext, xgT_ext, maskb_ext, wqkv_ext, wproj_ext,
          bproj_ext, out_ext):
    nc = tc.nc
    KP = ntk * P
    KP2 = KP // 2

    singles = ctx.enter_context(tc.tile_pool(name="singles", bufs=1))
    qkv_pool = ctx.enter_context(tc.tile_pool(name="qkv", bufs=3))
    pt_pool = ctx.enter_context(tc.tile_pool(name="pt", bufs=18))
    zb_pool = ctx.enter_context(tc.tile_pool(name="zb", bufs=2))
    out_pool = ctx.enter_context(tc.tile_pool(name="outp", bufs=2))
    ps_pool = ctx.enter_context(tc.tile_pool(name="ps", bufs=3, space="PSUM"))
    ps_av = ctx.enter_context(tc.tile_pool(name="ps_av", bufs=2, space="PSUM"))
    dram_pool = ctx.enter_context(tc.tile_pool(name="dram", bufs=2, space="DRAM"))

    # ---- startup-critical DMAs first: pair-0 weights and the h0 x-halves
    # race on separate queues; bias_bc (only needed at projection time)
    # and the later weight chunks follow ----
    maskb = singles.tile([P, ntk], F32)
    nc.sync.dma_start(out=maskb[:], in_=maskb_ext.rearrange("i p -> p i"))

    xgT = singles.tile([P, 2, KT, KP2], BF16)
    xgT_v = xgT_ext.rearrange("p (h k n) -> p h k n", h=2, k=KT)
    for h in range(2):
        nc.scalar.dma_start(out=xgT[:, h], in_=xgT_v[:, h])
    xT = singles.tile([P, 2, KT, 512], BF16)
    xT_v = xT_ext.rearrange("p (h k n) -> p h k n", h=2, k=KT)
    for h in range(2):
        nc.gpsimd.dma_start(out=xT[:, h], in_=xT_v[:, h])

    # weight chunks spread across all three queues by need-time so none
    # starves behind the x transfers: sync gets the early pairs, the x
    # queues pick up the later pairs once their halves are done
    wqkv_b = singles.tile([P, 3 * NPAIR, KT, P], BF16)
    wqkv_v = wqkv_ext.rearrange("p (m k j) -> p m k j", m=3 * NPAIR, k=KT)
    w_queue = {0: nc.sync, 1: nc.sync, 2: nc.sync,
               3: nc.scalar, 4: nc.gpsimd, 5: nc.scalar}
    for g in range(NPAIR):
        w_queue.get(g, nc.sync).dma_start(
            out=wqkv_b[:, 3 * g:3 * g + 3], in_=wqkv_v[:, 3 * g:3 * g + 3])
    bias_bc = singles.tile([P, C], F32)
    nc.gpsimd.dma_start(out=bias_bc[:],
                        in_=bproj_ext[0:1, :].to_broadcast([P, C]))
    wproj_sb = singles.tile([P, KT, C], BF16)
    nc.scalar.dma_start(
        out=wproj_sb[:], in_=wproj_ext.rearrange("p (k n) -> p k n", k=KT))

    ident_f = singles.tile([P, P], F32)
    make_identity(nc, ident_f[:])
    ident = singles.tile([P, P], BF16)
    nc.vector.tensor_copy(out=ident[:], in_=ident_f[:])
    zsel = singles.tile([P, DH], F32)
    nc.vector.memset(zsel[:], 0.0)
    nc.vector.memset(zsel[DH:DH + 1, :], 1.0)

    attnT = singles.tile([P, KT, N], BF16)

    # persistent [v | 1] tiles: ones column written once
    VE = DH + 1
    ve_tiles = [singles.tile([P, 2, VE], BF16, name=f"ve_s{i}")
                for i in range(ntk)]
    for t_ in ve_tiles:
        nc.vector.memset(t_[:, :, DH:DH + 1], 1.0)

    # ---- qkv projection units: u//2 -> (k, q, v), u%2 -> token half ----
    qkv_tiles = {}

    def qkv_unit(p, u):
        t, name = ((0, "k"), (1, "q"), (2, "v"))[u // 2]
        half = u % 2
        cols = N if name == "q" else KP
        w = cols // 2
        if (p, name) not in qkv_tiles:
            qkv_tiles[(p, name)] = qkv_pool.tile(
                [P, cols], BF16, tag=name, name=f"{name}{p}")
        dst = qkv_tiles[(p, name)]
        src = xT if name == "q" else xgT
        ps = ps_pool.tile([P, w], F32, tag="ps", name=f"ps_{name}{p}_{half}")
        for k in range(KT):
            nc.tensor.matmul(
                out=ps[:],
                lhsT=wqkv_b[:, 3 * p + t, k, :],
                rhs=src[:, half, k, :],
                start=(k == 0), stop=(k == KT - 1))
        nc.vector.tensor_copy(out=dst[:, half * w:(half + 1) * w], in_=ps[:])

    # pair 0's h0 units (and the h1 units pass A will need) run upfront;
    # everything else trickles in as PE filler via the unit fifo
    unit_fifo = deque()
    for u in (0, 4, 2, 1, 5):
        qkv_unit(0, u)
    unit_fifo.append((0, 3))

    def normalize(p, qc, av_sbs):
        # attnT[:, p, qc-half] = av * (1/normalizer-row)
        for hi in range(2):
            av_sb = av_sbs[hi]
            zrow = av_sb[DH:DH + 1, qc * 512:(qc + 1) * 512]
            if p == NPAIR - 1:
                # PE broadcast: fan the raw normalizer row out with a
                # selector matmul (row 64 of zsel is 1), then recip the
                # broadcast at partition 0 — no DMA on the critical path.
                # (reciprocal_approx_fast miscomputes at partition offset
                # 64, so the recip must come after the broadcast.)
                zps = ps_pool.tile([DH, 512], F32, tag="ps",
                                   name=f"zps{p}_{hi}_{qc}")
                nc.tensor.matmul(
                    out=zps[:], lhsT=zsel[0:DH + 1, :],
                    rhs=av_sb[0:DH + 1, qc * 512:(qc + 1) * 512],
                    start=True, stop=True)
                zb = zb_pool.tile([DH, 512], F32, tag="zb",
                                  name=f"zbp{p}_{hi}_{qc}")
                nc.vector.reciprocal_approx_fast(out=zb[:], in_=zps[:])
                zin = zb[:]
            else:
                # DRAM bounce, hidden under the following pass's PE work
                zdram = dram_pool.tile([1, 512], F32, tag="zdram",
                                       name=f"zd{p}_{hi}_{qc}")
                nc.sync.dma_start(out=zdram[:], in_=zrow)
                zb = zb_pool.tile([DH, 512], F32, tag="zb",
                                  name=f"zb{p}_{hi}_{qc}")
                nc.sync.dma_start(out=zb[:],
                                  in_=zdram[0:1, :].to_broadcast([DH, 512]))
                nc.vector.reciprocal_approx_fast(out=zb[:], in_=zb[:])
                zin = zb[:]
            nc.vector.scalar_tensor_tensor(
                out=attnT[64 * hi:64 * (hi + 1), p, qc * 512:(qc + 1) * 512],
                in0=av_sb[0:DH, qc * 512:(qc + 1) * 512], scalar=1.0, in1=zin,
                op0=ALU.mult, op1=ALU.mult)

    out_queues = [nc.sync, nc.scalar, nc.gpsimd]

    def proj_wave(m0, m1):
        # out rows m*128..: needs attnT q-columns m*128.. only
        chunks = [(0, 512), (512, 256)]
        for m in range(m0, m1):
            out_sb = out_pool.tile([P, C], BF16, tag="out_sb", name=f"out_sb{m}")
            for j, (lo, w) in enumerate(chunks):
                pps = ps_pool.tile([P, w], F32, tag="ps", name=f"ps_proj{m}_{j}")
                for k in range(KT):
                    nc.tensor.matmul(
                        out=pps[:],
                        lhsT=attnT[:, k, m * P:(m + 1) * P],
                        rhs=wproj_sb[:, k, lo:lo + w],
                        start=(k == 0), stop=(k == KT - 1))
                nc.vector.scalar_tensor_tensor(
                    out=out_sb[:, lo:lo + w], in0=pps[:], scalar=1.0,
                    in1=bias_bc[:, lo:lo + w], op0=ALU.mult, op1=ALU.add)
            out_queues[m % 3].dma_start(
                out=out_ext[m * P:(m + 1) * P, :], in_=out_sb[:])

    # ---- per head pair: two passes (q-half 0, then q-half 1) ----
    for p in range(NPAIR):
        if p + 1 < NPAIR:
            for u in (0, 2, 4, 1, 5, 3):
                unit_fifo.append((p + 1, u))
        nslots = 2 * ntk
        pending = len(unit_fifo)

        qt = qkv_tiles[(p, "q")]
        kt_ = qkv_tiles[(p, "k")]
        vt = qkv_tiles[(p, "v")]

        av = [ps_av.tile([VE, 512], F32, tag="ps_av", name=f"av{p}_{hi}")
              for hi in range(2)]
        av_sbs = [zb_pool.tile([VE, N], F32, tag="av_sb", name=f"avs{p}_{hi}")
                  for hi in range(2)]

        # this pair's own deferred units must precede the loop that
        # reads their output
        while unit_fifo and unit_fifo[0][0] == p:
            qkv_unit(*unit_fifo.popleft())

        all_pts = [[None, None] for _ in range(ntk)]
        # pair 0 runs q-half 0 for every key tile before touching q-half 1:
        # its pass A needs only the h0 DMAs, so attention starts ~5 us
        # earlier; later pairs interleave halves (better steady cadence)
        if p == 0:
            order = [(kb, 0) for kb in range(ntk)] + \
                    [(kb, 1) for kb in range(ntk)]
        else:
            order = [(kb, qc) for kb in range(ntk) for qc in range(2)]
        for kb, qc in order:
            if True:
                psq = ps_pool.tile([P, N], F32, tag="ps", name=f"ps_s{p}_{kb}_{qc}")
                for hi in range(2):
                    nc.tensor.matmul(
                        out=psq[:, 512 * hi:512 * (hi + 1)],
                        lhsT=kt_[64 * hi:64 * (hi + 1), kb * P:(kb + 1) * P],
                        rhs=qt[64 * hi:64 * (hi + 1), qc * 512:(qc + 1) * 512],
                        start=True, stop=True)
                pt = pt_pool.tile([P, N], BF16, tag="pt", name=f"pt{p}_{kb}_{qc}")
                nc.scalar.activation(
                    out=pt[:], in_=psq[:], func=AF.Exp,
                    bias=maskb[:, kb:kb + 1], scale=SCALE)
                all_pts[kb][qc] = pt

            if qc == 0:
                # v natural block for this key tile (needed later, so it
                # sits behind the scores on the PE queue)
                vnat = ps_pool.tile([P, P], BF16, tag="ps", name=f"vn{p}_{kb}")
                nc.tensor.transpose(
                    out=vnat[:], in_=vt[:, kb * P:(kb + 1) * P],
                    identity=ident[:])
                nc.vector.tensor_copy(
                    out=ve_tiles[kb][:, :, 0:DH],
                    in_=vnat[:].rearrange("p (h d) -> p h d", h=2))

                # previous key tile's qc0 p@v: slot-free PE work while
                # exps run
                if kb > 0:
                    for hi in range(2):
                        nc.tensor.matmul(
                            out=av[hi][:],
                            lhsT=ve_tiles[kb - 1][:, hi, :],
                            rhs=all_pts[kb - 1][0][:, 512 * hi:512 * (hi + 1)],
                            start=(kb - 1 == 0), stop=False)

            # PE filler while exps run: queued qkv units, spread evenly
            # (pair 0's loop stays clean — the queue is in-order and
            # fillers there would stall it on not-yet-landed h1 DMAs)
            if p > 0 and qc == 1:
                want = (pending * (kb + 1) + ntk - 1) // ntk
                while pending - len(unit_fifo) < want and unit_fifo:
                    qkv_unit(*unit_fifo.popleft())

        # close qc0 with the last key tile, copy out, normalize half 0
        for hi in range(2):
            nc.tensor.matmul(
                out=av[hi][:], lhsT=ve_tiles[ntk - 1][:, hi, :],
                rhs=all_pts[ntk - 1][0][:, 512 * hi:512 * (hi + 1)],
                start=(ntk == 1), stop=True)
        for hi in range(2):
            nc.vector.tensor_copy(out=av_sbs[hi][:, 0:512], in_=av[hi][:])
        normalize(p, 0, av_sbs)

        # qc1 p@v as one tight pass (reuses the av psum tiles)
        for kb in range(ntk):
            for hi in range(2):
                nc.tensor.matmul(
                    out=av[hi][:],
                    lhsT=ve_tiles[kb][:, hi, :],
                    rhs=all_pts[kb][1][:, 512 * hi:512 * (hi + 1)],
                    start=(kb == 0), stop=(kb == ntk - 1))
        for hi in range(2):
            nc.vector.tensor_copy(out=av_sbs[hi][:, 512:1024], in_=av[hi][:])
        normalize(p, 1, av_sbs)

        while unit_fifo:
            qkv_unit(*unit_fifo.popleft())

        if p == NPAIR - 1:
            # projection over q 0:511 only needs every pair's qc0 attnT;
            # it fills the PE while this pair's qc1 normalizer settles
            proj_wave(0, 4)
            proj_wave(4, 8)


def build(ntk):
    nc = bacc.Bacc()
    KP = ntk * P
    xT_ext = nc.declare_dram_parameter("xT", [P, KT * N], BF16, isOutput=False)
    xgT_ext = nc.declare_dram_parameter("xgT", [P, KT * KP], BF16, isOutput=False)
    maskb_ext = nc.declare_dram_parameter("maskb", [ntk, P], F32, isOutput=False)
    wqkv_ext = nc.declare_dram_parameter(
        "w_qkv", [P, 3 * NPAIR * KT * P], BF16, isOutput=False)
    wproj_ext = nc.declare_dram_parameter("w_proj", [P, KT * C], BF16,
                                          isOutput=False)
    bproj_ext = nc.declare_dram_parameter("b_proj", [1, C], F32, isOutput=False)
    out_ext = nc.declare_dram_parameter("out", [N, C], BF16, isOutput=True)

    with tile.TileContext(nc) as tc, ExitStack() as ctx:
        _body(ctx, tc, ntk, xT_ext.ap(), xgT_ext.ap(), maskb_ext.ap(),
              wqkv_ext.ap(), wproj_ext.ap(), bproj_ext.ap(), out_ext.ap())
    nc.finalize()
    return nc


_NC_CACHE = {}


def _get_nc(ntk):
    if ntk not in _NC_CACHE:
        _NC_CACHE[ntk] = build(ntk)
    return _NC_CACHE[ntk]


def _make_in_maps(inputs):
    x = np.ascontiguousarray(np.asarray(inputs["x"], dtype=np.float32))
    mask = np.ascontiguousarray(np.asarray(inputs["mask"], dtype=np.int32))
    w_qkv = np.ascontiguousarray(np.asarray(inputs["w_qkv"], dtype=np.float32))
    w_proj = np.ascontiguousarray(np.asarray(inputs["w_proj"], dtype=np.float32))
    b_proj = np.ascontiguousarray(
        np.asarray(inputs["b_proj"], dtype=np.float32)).reshape(1, C)

    # key compaction: gather unmasked key positions, pad to a tile multiple
    idxs = [np.nonzero(mask[b] == 0)[0] for b in range(B)]
    ntk = max(1, -(-max(len(i) for i in idxs) // P))
    KP = ntk * P

    # w_qkv [C, 3C] -> [P, pair*type*kchunk*128] bf16, type order (k, q, v)
    w3 = w_qkv.reshape(KT, P, 3, NPAIR, P)[:, :, [1, 0, 2], :, :]
    wq_u = np.ascontiguousarray(
        w3.transpose(1, 3, 2, 0, 4)).reshape(P, -1).astype(BF)
    wp_u = np.ascontiguousarray(
        w_proj.reshape(KT, P, C).transpose(1, 0, 2)).reshape(P, KT * C).astype(BF)

    maps = []
    for b in range(B):
        idx = idxs[b]
        nb = len(idx)
        pad = np.zeros(KP, np.int64)
        pad[:nb] = idx  # pad slots point anywhere; their bias kills them
        maskb_h = np.full(KP, MASK_NEG, np.float32)
        maskb_h[:nb] = 0.0
        xb = x[b]
        # [P, half, kchunk, tokens-in-half], partition-major contiguous
        xT_h = xb.T.reshape(KT, P, 2, 512).transpose(1, 2, 0, 3)
        xgT_h = xb[pad].T.reshape(KT, P, 2, KP // 2).transpose(1, 2, 0, 3)
        maps.append({
            "xT": np.ascontiguousarray(xT_h).reshape(P, KT * N).astype(BF),
            "xgT": np.ascontiguousarray(xgT_h).reshape(P, KT * KP).astype(BF),
            "maskb": maskb_h.reshape(ntk, P),
            "w_qkv": wq_u,
            "w_proj": wp_u,
            "b_proj": b_proj,
        })
    return maps, ntk


def _run(inputs, trace=False, **kwargs):
    in_maps, ntk = _make_in_maps(inputs)
    nc = _get_nc(ntk)
    res = run_bass_kernel_spmd(nc, in_maps, list(range(NCORES)), trace=trace,
                               **kwargs)
    out = np.stack([np.asarray(res.results[i]["out"]).astype(np.float32)
                    for i in range(NCORES)])
    return out, res


def kernel(**inputs):
    out, _ = _run(inputs)
    return out
